# revision 1
# baseline (speedup 1.0000x reference)
"""GMM log-likelihood kernel for Trainium2 (Bass/Tile), 8-core data-parallel.

Math (host precompute in f64):
  B_k = L_k^{-1} (Cholesky inverse),  w_k = B_k^T B_k mu_k
  wlp_k(x) = -0.5*||B_k x||^2 + w_k . x + C_k
  lse(x)   = m0 + log(sum_k exp(wlp_k - m0))   (m0 = global shift, safe:
             measured per-sample max wlp spread is ~37 nats << f32 exp range)
  out      = sum_x lse(x)

Per core: the [25000, 64] data slice (zero-padded to 196 tiles of 128
samples) is processed in pairs of tiles: PE transposes each pair into a
[128,128] stationary (two 64-row feature blocks), then row-packed bf16
matmuls against the replicated moving operand [B_all | W] produce
Y [128 samples, 1024] + lin [128, 16] per tile.  ACT squares Y out of
PSUM, DVE group-reduces the squares to per-component norms and assembles
wlp into a [128, 196*16] buffer.  A batched phase 2 does exp /
component-sum / log / masked accumulate, and a ones-matmul folds the 128
partitions into the final scalar.  Host sums the 8 per-core scalars.
"""

import numpy as np

N_COMPONENTS = 16
N_FEATURES = 64
N_SAMPLES = 200000
N_CORES = 8
PER_CORE = N_SAMPLES // N_CORES          # 25000
TILE_P = 128
N_TILES = -(-PER_CORE // TILE_P)         # 196 (ceil)
N_PAIRS = (N_TILES + 1) // 2             # 98
PADDED = N_TILES * TILE_P                # 25088
KD = N_COMPONENTS * N_FEATURES           # 1024

_CACHE = {}


def _build_nc(n_pairs):
    import concourse.tile as tile
    from concourse import bacc, mybir

    n_tiles = n_pairs * 2
    padded = n_tiles * TILE_P
    f32 = mybir.dt.float32
    bf16 = mybir.dt.bfloat16

    nc = bacc.Bacc("TRN2", target_bir_lowering=False, debug=False,
                   num_devices=N_CORES)

    xp = nc.dram_tensor("xp", [padded, N_FEATURES], bf16, kind="ExternalInput").ap()
    bmov2 = nc.dram_tensor("bmov2", [128, KD + N_COMPONENTS], bf16,
                           kind="ExternalInput").ap()
    cq = nc.dram_tensor("cq", [1, N_COMPONENTS], f32, kind="ExternalInput").ap()
    oner = nc.dram_tensor("oner", [1, 128], f32, kind="ExternalInput").ap()
    mask = nc.dram_tensor("mask", [128, n_tiles], f32, kind="ExternalInput").ap()
    ident = nc.dram_tensor("ident", [128, 128], bf16, kind="ExternalInput").ap()
    ones = nc.dram_tensor("ones", [128, 1], f32, kind="ExternalInput").ap()
    out = nc.dram_tensor("out", [1, 1], f32, kind="ExternalOutput").ap()

    W = n_tiles * N_COMPONENTS

    with tile.TileContext(nc) as tc:
        with (
            tc.tile_pool(name="const", bufs=1) as const_pool,
            tc.tile_pool(name="wbuf", bufs=1) as wbuf_pool,
            tc.tile_pool(name="xin", bufs=4) as xin_pool,
            tc.tile_pool(name="xt", bufs=3) as xt_pool,
            tc.tile_pool(name="ysq", bufs=2) as ysq_pool,
            tc.tile_pool(name="sm", bufs=4) as sm_pool,
            tc.tile_pool(name="tp", bufs=2, space="PSUM") as tp_pool,
            tc.tile_pool(name="yp", bufs=2, space="PSUM") as yp_pool,
            tc.tile_pool(name="lp", bufs=2, space="PSUM") as lp_pool,
        ):
            bm = const_pool.tile([128, KD + N_COMPONENTS], bf16)
            nc.sync.dma_start(bm[:], bmov2[:])
            cqs = const_pool.tile([1, N_COMPONENTS], f32)
            nc.sync.dma_start(cqs[:], cq[:])
            onr = const_pool.tile([1, 128], f32)
            nc.sync.dma_start(onr[:], oner[:])
            msks = const_pool.tile([128, n_tiles], f32)
            nc.sync.dma_start(msks[:], mask[:])
            idn = const_pool.tile([128, 128], bf16)
            nc.sync.dma_start(idn[:], ident[:])
            on1 = const_pool.tile([128, 1], f32)
            nc.sync.dma_start(on1[:], ones[:])

            wbuf = wbuf_pool.tile([128, W], f32)
            ebuf = wbuf_pool.tile([128, W], f32)

            for p in range(n_pairs):
                xpair = xin_pool.tile([128, 128], bf16, tag="xpair")
                r0 = (2 * p) * TILE_P
                nc.sync.dma_start(xpair[:, 0:64], xp[r0:r0 + 128, :])
                nc.sync.dma_start(xpair[:, 64:128], xp[r0 + 128:r0 + 256, :])

                tp = tp_pool.tile([128, 128], bf16, tag="tp")
                nc.tensor.transpose(tp[:], xpair[:], idn[:])
                xt = xt_pool.tile([128, 128], bf16, tag="xt")
                nc.scalar.copy(xt[:], tp[:])

                ysq = ysq_pool.tile([128, 2 * KD], f32, tag="ysq")
                lps = []
                for h in range(2):
                    hp = h * 64
                    yp = yp_pool.tile([128, KD], f32, tag="yp")
                    lp = lp_pool.tile([128, N_COMPONENTS], f32, tag="lp")
                    lhs = xt[hp:hp + 64, :]
                    nc.tensor.matmul(yp[:, 0:512], lhs, bm[hp:hp + 64, 0:512])
                    nc.tensor.matmul(yp[:, 512:1024], lhs, bm[hp:hp + 64, 512:1024])
                    nc.tensor.matmul(lp[:], lhs, bm[hp:hp + 64, 1024:1040],
                                     start=True, stop=False)
                    nc.tensor.matmul(lp[:], onr[:], cqs[:],
                                     start=False, stop=True)
                    nc.scalar.activation(ysq[:, h * KD:(h + 1) * KD], yp[:],
                                         mybir.ActivationFunctionType.Square)
                    lps.append(lp)

                st = sm_pool.tile([128, 2 * N_COMPONENTS], f32, tag="st")
                nc.vector.reduce_sum(
                    st[:],
                    ysq[:].rearrange("p (k i) -> p k i", i=N_FEATURES),
                    axis=mybir.AxisListType.X)

                for h in range(2):
                    col = (2 * p + h) * N_COMPONENTS
                    nc.vector.scalar_tensor_tensor(
                        wbuf[:, col:col + N_COMPONENTS],
                        st[:, h * N_COMPONENTS:(h + 1) * N_COMPONENTS],
                        -0.5, lps[h][:],
                        op0=mybir.AluOpType.mult, op1=mybir.AluOpType.add)

            # phase 2
            nc.scalar.activation(ebuf[:], wbuf[:],
                                 mybir.ActivationFunctionType.Exp)
            rsum = const_pool.tile([128, n_tiles], f32)
            nc.vector.reduce_sum(
                rsum[:],
                ebuf[:].rearrange("p (t k) -> p t k", k=N_COMPONENTS),
                axis=mybir.AxisListType.X)
            lnr = const_pool.tile([128, n_tiles], f32)
            nc.scalar.activation(lnr[:], rsum[:],
                                 mybir.ActivationFunctionType.Ln)
            msum = const_pool.tile([128, n_tiles], f32)
            nc.vector.tensor_mul(msum[:], lnr[:], msks[:])
            csum = const_pool.tile([128, 1], f32)
            nc.vector.reduce_sum(csum[:], msum[:], axis=mybir.AxisListType.X)

            rp = tp_pool.tile([1, 1], f32, tag="tp")
            nc.tensor.matmul(rp[:], on1[:], csum[:])
            res = const_pool.tile([1, 1], f32)
            nc.scalar.copy(res[:], rp[:])
            nc.sync.dma_start(out[:], res[:])

    nc.compile()
    return nc


def _precompute(weights, means, covariances):
    """Host-side O(K d^3) prep in float64. Returns (bmov2, cq_row, m0)."""
    import ml_dtypes

    K, d = means.shape
    L = np.linalg.cholesky(covariances.astype(np.float64))
    half_logdet = np.log(np.diagonal(L, axis1=-2, axis2=-1)).sum(-1)
    eye = np.eye(d)
    B = np.stack([np.linalg.solve(L[k], eye) for k in range(K)])  # L^-1
    mu = means.astype(np.float64)
    c = np.einsum('kij,kj->ki', B, mu)
    w_lin = np.einsum('kij,ki->kj', B, c)
    r = (c * c).sum(-1)
    const = (np.log(weights.astype(np.float64))
             - 0.5 * d * np.log(2.0 * np.pi) - half_logdet)
    C = const - 0.5 * r
    m0 = float(C.max()) - 20.0

    bmov = np.zeros((d, K * d + K), np.float32)
    for k in range(K):
        bmov[:, k * d:(k + 1) * d] = B[k].T.astype(np.float32)
    bmov[:, K * d:] = w_lin.T.astype(np.float32)
    bmov2 = np.vstack([bmov, bmov]).astype(ml_dtypes.bfloat16)   # [128, 1040]
    cq_row = (C - m0).astype(np.float32)                         # [16]
    return bmov2, cq_row, m0


def _make_inputs(data, bmov2, cq_row, n_tiles):
    """Build the 8 per-core input maps for the padded per-core data slices."""
    import ml_dtypes

    padded = n_tiles * TILE_P
    cq = cq_row[None, :].astype(np.float32)
    oner = np.ones((1, 128), np.float32)
    mask = np.zeros((128, n_tiles), np.float32)
    for t in range(n_tiles):
        v = min(max(PER_CORE - t * TILE_P, 0), TILE_P)
        mask[:v, t] = 1.0
    ident = np.eye(128, dtype=ml_dtypes.bfloat16)
    ones = np.ones((128, 1), np.float32)

    in_maps = []
    for c in range(N_CORES):
        sl = data[c * PER_CORE:(c + 1) * PER_CORE]
        xp = np.zeros((padded, N_FEATURES), ml_dtypes.bfloat16)
        xp[:sl.shape[0]] = sl.astype(ml_dtypes.bfloat16)
        in_maps.append({"xp": xp, "bmov2": bmov2, "cq": cq, "mask": mask,
                        "ident": ident, "ones": ones, "oner": oner})
    return in_maps


def _run(data, weights, means, covariances, trace=False):
    from concourse.bass_utils import run_bass_kernel_spmd

    data = np.asarray(data, np.float32)
    bmov2, cq_row, m0 = _precompute(np.asarray(weights), np.asarray(means),
                                    np.asarray(covariances))
    if "nc" not in _CACHE:
        _CACHE["nc"] = _build_nc(N_PAIRS)
    nc = _CACHE["nc"]

    in_maps = _make_inputs(data, bmov2, cq_row, N_TILES)
    res = run_bass_kernel_spmd(nc, in_maps, list(range(N_CORES)), trace=trace)
    total = 0.0
    for c in range(N_CORES):
        total += float(res.results[c]["out"][0, 0]) + PER_CORE * m0
    return np.float32(total), res


def kernel(data, weights, means, covariances):
    return _run(data, weights, means, covariances)[0]



# revision 11
# speedup vs baseline: 1.3268x; 1.3268x over previous
"""GMM log-likelihood kernel for Trainium2 (Bass/Tile), 8-core data-parallel.

v2 design. Math (host precompute in f64):
  B_k = L_k^{-1},  w_k = B_k^T B_k mu_k
  wlp_k(x) = -0.5*||B_k x||^2 + w_k.x + C_k     (C_k absorbs logdet, log w, mu-term)
  S_k(x)   = ||B_k x||^2 - 2 w_k.x - 2(C_k - m0)   -> wlp - m0 = -S/2
  out      = sum_x [ m0 + log sum_k exp(-S_k/2) ]

Device dataflow (per core, 25088 padded samples = 196 tiles of 128):
  Host ships xall [65, 25088] fp16 (x^T plus a ones row) - no PE transposes.
  Per tile: ONE stationary load (xall column slice [65,128]); matmuls stream
  bmov [65, 1040] fp16 = [B-blocks | -2w/-2C] giving yp [128,1024] f32 and
  lp [128,16] f32 in PSUM. ACT squares yp into a 65-strided fp16 buffer,
  GpSimd copies lp into the 65th slot, DVE group-reduces [128,16,65] -> S f32.
  Batched phase 2: exp(-S/2) on ACT, component-sum + ln + masked accumulate,
  ones-matmul folds partitions, host adds 25000*m0 per core and sums cores.
"""

import numpy as np

N_COMPONENTS = 16
N_FEATURES = 64
N_SAMPLES = 200000
N_CORES = 8
PER_CORE = N_SAMPLES // N_CORES          # 25000
TILE_P = 128
N_TILES = -(-PER_CORE // TILE_P)         # 196
PADDED = N_TILES * TILE_P                # 25088
KD = N_COMPONENTS * N_FEATURES           # 1024
GW = N_FEATURES + 1                      # 65: squares + lp slot per component

BENCH = False   # add microbenchmark appendix instructions (for rate measuring)

_CACHE = {}


def _build_nc(bench=BENCH):
    import concourse.tile as tile
    from concourse import bacc, mybir

    f32 = mybir.dt.float32
    f16 = mybir.dt.float16

    nc = bacc.Bacc("TRN2", target_bir_lowering=False, debug=False,
                   num_devices=N_CORES)

    xall = nc.dram_tensor("xall", [GW, PADDED], f16, kind="ExternalInput").ap()
    bmov = nc.dram_tensor("bmov", [GW, KD + N_COMPONENTS], f16,
                          kind="ExternalInput").ap()
    mask = nc.dram_tensor("mask", [128, N_TILES], f32, kind="ExternalInput").ap()
    ones = nc.dram_tensor("ones", [128, 1], f32, kind="ExternalInput").ap()
    out = nc.dram_tensor("out", [1, 1], f32, kind="ExternalOutput").ap()
    if bench:
        bout = nc.dram_tensor("bout", [128, 64], f32, kind="ExternalOutput").ap()

    n_chunks = 8
    chunk = PADDED // n_chunks            # 3136 cols

    with tile.TileContext(nc) as tc:
        with (
            tc.tile_pool(name="const", bufs=1) as const_pool,
            tc.tile_pool(name="ysq", bufs=4) as ysq_pool,
            tc.tile_pool(name="yp", bufs=2, space="PSUM") as yp_pool,
            tc.tile_pool(name="lp", bufs=1, space="PSUM") as lp_pool,
            tc.tile_pool(name="rp", bufs=1, space="PSUM") as rp_pool,
        ):
            xs = const_pool.tile([GW, PADDED], f16)
            dma_engines = [nc.sync, nc.gpsimd, nc.sync, nc.gpsimd]
            for c in range(n_chunks):
                eng = dma_engines[c % len(dma_engines)]
                sl = slice(c * chunk, (c + 1) * chunk)
                eng.dma_start(xs[:, sl], xall[:, sl])
            bm = const_pool.tile([GW, KD + N_COMPONENTS], f16)
            nc.sync.dma_start(bm[:], bmov[:])
            msks = const_pool.tile([128, N_TILES], f32)
            nc.sync.dma_start(msks[:], mask[:])
            on1 = const_pool.tile([128, 1], f32)
            nc.sync.dma_start(on1[:], ones[:])

            sbuf_S = const_pool.tile([128, N_TILES * N_COMPONENTS], f32)

            # two persistent lp banks; tile t -> bank t%2, slot (t//2)%16
            lp_banks = []
            for b in range(2):
                lpb = lp_pool.tile([128, 512], f32, tag=f"lpb{b}", name=f"lpb{b}")
                lp_banks.append(lpb)

            for t in range(N_TILES):
                lhs = xs[:, t * TILE_P:(t + 1) * TILE_P]
                yp = yp_pool.tile([128, KD], f32, tag="yp")
                nc.tensor.matmul(yp[:, 0:512], lhs, bm[:, 0:512])
                nc.tensor.matmul(yp[:, 512:1024], lhs, bm[:, 512:1024])
                lpb = lp_banks[t % 2]
                s0 = ((t // 2) % 16) * 16
                lslot = lpb[:, s0:s0 + 16]
                nc.tensor.matmul(lslot, lhs, bm[:, KD:KD + N_COMPONENTS])

                ysq = ysq_pool.tile([128, N_COMPONENTS * GW], f16, tag="ysq")
                ysq_v = ysq[:].rearrange("p (k i) -> p k i", i=GW)
                nc.scalar.activation(ysq_v[:, :, 0:64], yp[:],
                                     mybir.ActivationFunctionType.Square)
                nc.scalar.copy(ysq_v[:, :, 64:65],
                               lslot.rearrange("p (k i) -> p k i", i=1))
                nc.vector.reduce_sum(
                    sbuf_S[:, t * N_COMPONENTS:(t + 1) * N_COMPONENTS],
                    ysq_v, axis=mybir.AxisListType.X)

            # phase 2 (batched)
            ebuf = const_pool.tile([128, N_TILES * N_COMPONENTS], f32)
            nc.scalar.activation(ebuf[:], sbuf_S[:],
                                 mybir.ActivationFunctionType.Exp, scale=-0.5)
            esum = const_pool.tile([128, N_TILES], f32)
            nc.vector.reduce_sum(
                esum[:], ebuf[:].rearrange("p (t k) -> p t k", k=N_COMPONENTS),
                axis=mybir.AxisListType.X)
            lnr = const_pool.tile([128, N_TILES], f32)
            nc.scalar.activation(lnr[:], esum[:],
                                 mybir.ActivationFunctionType.Ln)
            msum = const_pool.tile([128, N_TILES], f32)
            nc.vector.tensor_tensor(msum[:], lnr[:], msks[:],
                                    op=mybir.AluOpType.mult)
            csum = const_pool.tile([128, 1], f32)
            nc.vector.reduce_sum(csum[:], msum[:], axis=mybir.AxisListType.X)

            rp = rp_pool.tile([1, 1], f32, tag="rp")
            nc.tensor.matmul(rp[:], on1[:], csum[:])
            res = const_pool.tile([1, 1], f32)
            nc.scalar.copy(res[:], rp[:])
            nc.sync.dma_start(out[:], res[:])

            if bench:
                _bench_appendix(nc, tc, const_pool, yp_pool, bout, mybir)

    nc.compile()
    return nc


def _bench_appendix(nc, tc, pool, yp_pool, bout, mybir):
    """Throwaway instruction sequences to measure HW rates from the NTFF."""
    f32 = mybir.dt.float32
    f16 = mybir.dt.float16
    xin = pool.tile([128, 1040], f16)
    nc.gpsimd.memset(xin[:], 1.0)
    sc32 = pool.tile([128, 64], f32)
    sc16a = pool.tile([128, 1040], f16)
    sc16b = pool.tile([128, 16], f16)

    with nc.allow_low_precision("bench"):
        # a) grouped reduce fp16->fp16
        for _ in range(8):
            nc.vector.reduce_sum(sc16b[:],
                                 xin[:].rearrange("p (k i) -> p k i", i=65),
                                 axis=mybir.AxisListType.X)

        # c) tensor_tensor fp16 (2x check)
        for _ in range(8):
            nc.vector.tensor_tensor(sc16a[:, 0:512], xin[:, 0:512],
                                    xin[:, 512:1024], op=mybir.AluOpType.mult)
        # d) custom DVE op: TENSOR_TENSOR_REDUCE fp16 (custom-op rate)
        acc = pool.tile([128, 1], f32)
        for _ in range(8):
            nc.vector.tensor_tensor_reduce(
                sc16a[:, 0:1024], xin[:, 0:1024], xin[:, 0:1024], 1.0, 0.0,
                op0=mybir.AluOpType.mult, op1=mybir.AluOpType.add,
                accum_out=acc[:])
        bst = pool.tile([128, 48], f32)
        nc.gpsimd.memset(bst[:], 0.0)
        # f) 512-col matmuls, back-to-back same stationary
        wst = pool.tile([65, 128], f16)
        nc.gpsimd.memset(wst[:], 0.5)
        wmv = pool.tile([65, 1024], f16)
        nc.gpsimd.memset(wmv[:], 0.5)
        for _ in range(4):
            ypw = yp_pool.tile([128, 1024], f32, tag="yp")
            nc.tensor.matmul(ypw[:, 0:512], wst[:], wmv[:, 0:512])
            nc.tensor.matmul(ypw[:, 512:1024], wst[:], wmv[:, 512:1024])
            nc.scalar.activation(sc16a[:, 0:1024], ypw[:],
                                 mybir.ActivationFunctionType.Square)
    nc.vector.reduce_sum(sc32[:, 0:16],
                         sc16a[:, 0:1024].rearrange("p (k i) -> p k i", i=64),
                         axis=mybir.AxisListType.X)
    nc.vector.tensor_copy(sc32[:, 16:32], bst[:, 0:16])
    nc.vector.tensor_copy(sc32[:, 32:33], acc[:])
    nc.vector.tensor_copy(sc32[:, 33:49], sc16b[:])
    nc.sync.dma_start(bout[:], sc32[:, 0:64].rearrange("p f -> p f"))


def _precompute(weights, means, covariances):
    """Host-side O(K d^3) prep in float64. Returns (bmov, m0)."""
    import ml_dtypes

    K, d = means.shape
    L = np.linalg.cholesky(covariances.astype(np.float64))
    half_logdet = np.log(np.diagonal(L, axis1=-2, axis2=-1)).sum(-1)
    eye = np.eye(d)
    B = np.stack([np.linalg.solve(L[k], eye) for k in range(K)])  # L^-1
    mu = means.astype(np.float64)
    c = np.einsum('kij,kj->ki', B, mu)
    w_lin = np.einsum('kij,ki->kj', B, c)
    r = (c * c).sum(-1)
    const = (np.log(weights.astype(np.float64))
             - 0.5 * d * np.log(2.0 * np.pi) - half_logdet)
    C = const - 0.5 * r
    m0 = float(C.max()) - 20.0

    bmov = np.zeros((GW, KD + N_COMPONENTS), np.float64)
    for k in range(K):
        bmov[0:d, k * d:(k + 1) * d] = B[k].T
    bmov[0:d, KD:] = (-2.0 * w_lin).T
    bmov[d, KD:] = -2.0 * (C - m0)
    return bmov.astype(np.float16), m0


def _make_inputs(data, bmov):
    import ml_dtypes

    mask = np.zeros((128, N_TILES), np.float32)
    for t in range(N_TILES):
        v = min(max(PER_CORE - t * TILE_P, 0), TILE_P)
        mask[:v, t] = 1.0
    ones = np.ones((128, 1), np.float32)

    d16 = data.astype(np.float16)
    in_maps = []
    for c in range(N_CORES):
        sl = d16[c * PER_CORE:(c + 1) * PER_CORE]
        xall = np.zeros((GW, PADDED), np.float16)
        xall[0:N_FEATURES, 0:PER_CORE] = sl.T
        xall[N_FEATURES, :] = 1.0
        in_maps.append({"xall": xall, "bmov": bmov, "mask": mask,
                        "ones": ones})
    return in_maps


def _run(data, weights, means, covariances, trace=False):
    from concourse.bass_utils import run_bass_kernel_spmd

    data = np.asarray(data, np.float32)
    bmov, m0 = _precompute(np.asarray(weights), np.asarray(means),
                           np.asarray(covariances))
    if "nc" not in _CACHE:
        _CACHE["nc"] = _build_nc()
    nc = _CACHE["nc"]

    in_maps = _make_inputs(data, bmov)
    res = run_bass_kernel_spmd(nc, in_maps, list(range(N_CORES)), trace=trace)
    total = 0.0
    for c in range(N_CORES):
        total += float(res.results[c]["out"][0, 0]) + PER_CORE * m0
    return np.float32(total), res


def kernel(data, weights, means, covariances):
    return _run(data, weights, means, covariances)[0]


# revision 12
# speedup vs baseline: 1.5519x; 1.1696x over previous
"""GMM log-likelihood kernel for Trainium2 (Bass/Tile), 8-core data-parallel.

v3 design. Math (host precompute in f64):
  B_k = L_k^{-1},  w_k = B_k^T B_k mu_k
  wlp_k(x) = -0.5*||B_k x||^2 + w_k.x + C_k     (C_k absorbs logdet, log w, mu-term)
  S_k(x)   = ||B_k x||^2 - 2 w_k.x - 2(C_k - m0)   -> wlp - m0 = -S/2
  out      = sum_x [ m0 + log sum_k exp(-S_k/2) ]

Device dataflow (per core, 25088 padded samples = 196 tiles of 128, grouped
in 8-tile blocks):
  Host ships xall [65, 25088] fp16 (x^T plus a ones row) - no PE transposes.
  Per tile: ONE stationary load (xall column slice [65,128]); matmuls stream
  bmov [65, 1040] fp16 = [B-blocks | -2w/-2C] giving yp [128,1024] f32 and a
  16-wide lp slot (8 consecutive tiles share one PSUM lp bank).  ACT squares
  yp into the 65-strided slots of an 8-tile mega buffer.  Once per 8-tile
  group, ACT copies the whole lp bank [128,128] into the 65th slots, and DVE
  runs ONE grouped reduce [128, 8*16, 65] -> S f32.  Batched phase 2:
  exp(-S/2) on ACT, component-sum + ln + masked accumulate, ones-matmul
  folds partitions; host adds 25000*m0 per core and sums cores.
"""

import numpy as np

N_COMPONENTS = 16
N_FEATURES = 64
N_SAMPLES = 200000
N_CORES = 8
PER_CORE = N_SAMPLES // N_CORES          # 25000
TILE_P = 128
N_TILES = -(-PER_CORE // TILE_P)         # 196
PADDED = N_TILES * TILE_P                # 25088
KD = N_COMPONENTS * N_FEATURES           # 1024
GW = N_FEATURES + 1                      # 65: squares + lp slot per component
GRP = 8                                  # tiles per reduce/evac group
N_GRP = -(-N_TILES // GRP)               # 25 (24 full + one of 4)

_CACHE = {}


def _build_nc():
    import concourse.tile as tile
    from concourse import bacc, mybir

    f32 = mybir.dt.float32
    f16 = mybir.dt.float16

    nc = bacc.Bacc("TRN2", target_bir_lowering=False, debug=False,
                   num_devices=N_CORES)

    xall = nc.dram_tensor("xall", [GW, PADDED], f16, kind="ExternalInput").ap()
    bmov = nc.dram_tensor("bmov", [GW, KD + N_COMPONENTS], f16,
                          kind="ExternalInput").ap()
    mask = nc.dram_tensor("mask", [128, N_TILES], f32, kind="ExternalInput").ap()
    ones = nc.dram_tensor("ones", [128, 1], f32, kind="ExternalInput").ap()
    out = nc.dram_tensor("out", [1, 1], f32, kind="ExternalOutput").ap()

    n_chunks = 8
    chunk = PADDED // n_chunks            # 3136 cols

    with tile.TileContext(nc) as tc:
        with (
            tc.tile_pool(name="const", bufs=1) as const_pool,
            tc.tile_pool(name="ysq", bufs=2) as ysq_pool,
            tc.tile_pool(name="yp", bufs=2, space="PSUM") as yp_pool,
            tc.tile_pool(name="lp", bufs=1, space="PSUM") as lp_pool,
            tc.tile_pool(name="rp", bufs=1, space="PSUM") as rp_pool,
        ):
            xs = const_pool.tile([GW, PADDED], f16)
            dma_engines = [nc.sync, nc.gpsimd, nc.sync, nc.gpsimd]
            for c in range(n_chunks):
                eng = dma_engines[c % len(dma_engines)]
                sl = slice(c * chunk, (c + 1) * chunk)
                eng.dma_start(xs[:, sl], xall[:, sl])
            bm = const_pool.tile([GW, KD + N_COMPONENTS], f16)
            nc.sync.dma_start(bm[:], bmov[:])
            msks = const_pool.tile([128, N_TILES], f32)
            nc.sync.dma_start(msks[:], mask[:])
            on1 = const_pool.tile([128, 1], f32)
            nc.sync.dma_start(on1[:], ones[:])

            sbuf_S = const_pool.tile([128, N_TILES * N_COMPONENTS], f32)

            # two lp PSUM banks; 8-tile group g -> bank g%2, slot j*16
            lp_banks = []
            for b in range(2):
                lpb = lp_pool.tile([128, 512], f32, tag=f"lpb{b}", name=f"lpb{b}")
                lp_banks.append(lpb)

            for g in range(N_GRP):
                gsz = min(GRP, N_TILES - g * GRP)
                ysq = ysq_pool.tile([128, GRP * N_COMPONENTS * GW], f16,
                                    tag="ysq")
                ysq_v = ysq[:].rearrange("p (t k i) -> p t k i",
                                         k=N_COMPONENTS, i=GW)
                lpb = lp_banks[g % 2]
                for j in range(gsz):
                    t = g * GRP + j
                    lhs = xs[:, t * TILE_P:(t + 1) * TILE_P]
                    yp = yp_pool.tile([128, KD], f32, tag="yp")
                    nc.tensor.matmul(yp[:, 0:512], lhs, bm[:, 0:512])
                    nc.tensor.matmul(yp[:, 512:1024], lhs, bm[:, 512:1024])
                    nc.tensor.matmul(lpb[:, j * 16:(j + 1) * 16], lhs,
                                     bm[:, KD:KD + N_COMPONENTS])
                    nc.scalar.activation(ysq_v[:, j, :, 0:64], yp[:],
                                         mybir.ActivationFunctionType.Square)
                # batched lp evac into the 65th slots
                nc.scalar.copy(
                    ysq_v[:, 0:gsz, :, 64:65],
                    lpb[:, 0:gsz * 16].rearrange("p (t k i) -> p t k i",
                                                 k=N_COMPONENTS, i=1))
                # one grouped reduce for the whole group
                nc.vector.reduce_sum(
                    sbuf_S[:, g * GRP * N_COMPONENTS:
                           (g * GRP + gsz) * N_COMPONENTS],
                    ysq_v[:, 0:gsz], axis=mybir.AxisListType.X)

            # phase 2 (batched)
            ebuf = const_pool.tile([128, N_TILES * N_COMPONENTS], f32)
            nc.scalar.activation(ebuf[:], sbuf_S[:],
                                 mybir.ActivationFunctionType.Exp, scale=-0.5)
            esum = const_pool.tile([128, N_TILES], f32)
            nc.vector.reduce_sum(
                esum[:], ebuf[:].rearrange("p (t k) -> p t k", k=N_COMPONENTS),
                axis=mybir.AxisListType.X)
            lnr = const_pool.tile([128, N_TILES], f32)
            nc.scalar.activation(lnr[:], esum[:],
                                 mybir.ActivationFunctionType.Ln)
            msum = const_pool.tile([128, N_TILES], f32)
            nc.vector.tensor_tensor(msum[:], lnr[:], msks[:],
                                    op=mybir.AluOpType.mult)
            csum = const_pool.tile([128, 1], f32)
            nc.vector.reduce_sum(csum[:], msum[:], axis=mybir.AxisListType.X)

            rp = rp_pool.tile([1, 1], f32, tag="rp")
            nc.tensor.matmul(rp[:], on1[:], csum[:])
            res = const_pool.tile([1, 1], f32)
            nc.scalar.copy(res[:], rp[:])
            nc.sync.dma_start(out[:], res[:])

    nc.compile()
    return nc


def _precompute(weights, means, covariances):
    """Host-side O(K d^3) prep in float64. Returns (bmov, m0)."""
    K, d = means.shape
    L = np.linalg.cholesky(covariances.astype(np.float64))
    half_logdet = np.log(np.diagonal(L, axis1=-2, axis2=-1)).sum(-1)
    eye = np.eye(d)
    B = np.stack([np.linalg.solve(L[k], eye) for k in range(K)])  # L^-1
    mu = means.astype(np.float64)
    c = np.einsum('kij,kj->ki', B, mu)
    w_lin = np.einsum('kij,ki->kj', B, c)
    r = (c * c).sum(-1)
    const = (np.log(weights.astype(np.float64))
             - 0.5 * d * np.log(2.0 * np.pi) - half_logdet)
    C = const - 0.5 * r
    m0 = float(C.max()) - 20.0

    bmov = np.zeros((GW, KD + N_COMPONENTS), np.float64)
    for k in range(K):
        bmov[0:d, k * d:(k + 1) * d] = B[k].T
    bmov[0:d, KD:] = (-2.0 * w_lin).T
    bmov[d, KD:] = -2.0 * (C - m0)
    return bmov.astype(np.float16), m0


def _make_inputs(data, bmov):
    mask = np.zeros((128, N_TILES), np.float32)
    for t in range(N_TILES):
        v = min(max(PER_CORE - t * TILE_P, 0), TILE_P)
        mask[:v, t] = 1.0
    ones = np.ones((128, 1), np.float32)

    d16 = data.astype(np.float16)
    in_maps = []
    for c in range(N_CORES):
        sl = d16[c * PER_CORE:(c + 1) * PER_CORE]
        xall = np.zeros((GW, PADDED), np.float16)
        xall[0:N_FEATURES, 0:PER_CORE] = sl.T
        xall[N_FEATURES, :] = 1.0
        in_maps.append({"xall": xall, "bmov": bmov, "mask": mask,
                        "ones": ones})
    return in_maps


def _run(data, weights, means, covariances, trace=False):
    from concourse.bass_utils import run_bass_kernel_spmd

    data = np.asarray(data, np.float32)
    bmov, m0 = _precompute(np.asarray(weights), np.asarray(means),
                           np.asarray(covariances))
    if "nc" not in _CACHE:
        _CACHE["nc"] = _build_nc()
    nc = _CACHE["nc"]

    in_maps = _make_inputs(data, bmov)
    res = run_bass_kernel_spmd(nc, in_maps, list(range(N_CORES)), trace=trace)
    total = 0.0
    for c in range(N_CORES):
        total += float(res.results[c]["out"][0, 0]) + PER_CORE * m0
    return np.float32(total), res


def kernel(data, weights, means, covariances):
    return _run(data, weights, means, covariances)[0]
